# revision 1
# baseline (speedup 1.0000x reference)
"""Decoder self-attention on 8 TRN2 NeuronCores.

Sharding: data-parallel over batch (2) x tensor-parallel over heads (4 groups
of 4 heads).  Core c handles batch c//4, heads 4*(c%4) .. 4*(c%4)+3.
Each core computes q/k/v projections for its head group, causal-masked
softmax attention, and its partial contribution ctx_g @ wo_g.  The host sums
the 4 partials per batch (row-parallel wo reduction done host-side).

All matmuls run in fp32r (TRN2 reduced-precision fp32, ~1e-4 rel err, full
PE rate at moving-dim >= 256).

Device data layout is fully transposed:
  xT   [H=1024, L]      input.T          (host pre-transposes)
  qT,kT [128, 2, L]     (wq cols chunk c2) x L, partitions = projection dim
  S.T  [lk 128, lq]     logit strips in PSUM
  expS [lk 128, lq]     exp'd strips in SBUF (fp32r)
  v_aug [128, lt, 4*65] v rows + ones column per head (PV also produces rowsum)
  ctxT [128, 2, L]      normalized context, partitions = ctx dim
  out  [L, 1024]        natural layout partial output
"""

import os
from contextlib import ExitStack

import numpy as np

import concourse.bass as bass
import concourse.tile as tile
from concourse import bacc, mybir
from concourse.bass_utils import run_bass_kernel_spmd

f32 = mybir.dt.float32
f32r = mybir.dt.float32r

H = 1024          # hidden dim
WG = 256          # weight-column group per core (4 heads x 64)
NH = 4            # heads per core
HD = 64           # head dim
INV_SQRT_D = 1.0 / 32.0  # 1/sqrt(1024)

_PROGRAM_CACHE = {}
LAST_RESULT = None


def build_program(L=2048, QCH=1024, repeat=1, opts=None):
    """Build the per-core bass program (identical on all cores).

    repeat>1 replicates the whole compute body (timing instrumentation:
    one dispatch executes the body N times back-to-back)."""
    assert L % QCH == 0 and QCH % 512 == 0 and QCH <= 1024
    opts = dict(opts or {})
    NLT = L // 128      # number of 128-row l tiles
    NQH = L // QCH      # number of q chunks
    KPQ = QCH // 128    # k-tiles per q chunk

    nc = bacc.Bacc("TRN2", target_bir_lowering=False, debug=False)
    xT_d = nc.dram_tensor("xT", [H, L], f32r, kind="ExternalInput").ap()
    wq_d = nc.dram_tensor("wq", [H, WG], f32r, kind="ExternalInput").ap()
    wk_d = nc.dram_tensor("wk", [H, WG], f32r, kind="ExternalInput").ap()
    wv_d = nc.dram_tensor("wv", [H, WG], f32r, kind="ExternalInput").ap()
    wo_d = nc.dram_tensor("wo", [WG, H], f32r, kind="ExternalInput").ap()
    lnm_d = nc.dram_tensor("lnm", [L], f32, kind="ExternalInput").ap()
    tri_d = nc.dram_tensor("tri", [128, 128], f32, kind="ExternalInput").ap()
    onescol_d = nc.dram_tensor(
        "onescol", [128, NLT * NH], f32r, kind="ExternalInput"
    ).ap()
    out_d = nc.dram_tensor("out", [L, H], f32, kind="ExternalOutput").ap()

    with ExitStack() as ctx:
        tc = ctx.enter_context(tile.TileContext(nc))

        # ---- persistent pools (live across phases) ----
        persist = ctx.enter_context(tc.tile_pool(name="persist", bufs=1))
        qT = persist.tile([128, 2, L], f32r, tag="qT")
        kT = persist.tile([128, 2, L], f32r, tag="kT")
        v_aug = persist.tile([128, NLT, NH * (HD + 1)], f32r, tag="vaug")
        ctxT0 = persist.tile([128, L], f32r, tag="ctxT0")
        ctxT1 = persist.tile([128, L], f32r, tag="ctxT1")
        ctxT = [ctxT0, ctxT1]
        wo_sb = persist.tile([128, 2, H], f32r, tag="wo")
        lnm_sb = persist.tile([128, NLT], f32, tag="lnm")
        tri_sb = persist.tile([128, 128], f32, tag="tri")


        env = locals()
        for _rep in range(repeat):
            _build_body(nc, tc, ctx, env)

    nc.compile()
    return nc


def _build_body(nc, tc, ctx, env):
    L = env["L"]; QCH = env["QCH"]; NLT = env["NLT"]; NQH = env["NQH"]
    xT_d = env["xT_d"]; wq_d = env["wq_d"]; wk_d = env["wk_d"]
    wv_d = env["wv_d"]; wo_d = env["wo_d"]; lnm_d = env["lnm_d"]
    tri_d = env["tri_d"]; onescol_d = env["onescol_d"]; out_d = env["out_d"]
    qT = env["qT"]; kT = env["kT"]; v_aug = env["v_aug"]; ctxT = env["ctxT"]
    wo_sb = env["wo_sb"]; lnm_sb = env["lnm_sb"]; tri_sb = env["tri_sb"]
    opts = env["opts"]
    GATE = opts.get("gate", 2)
    EXPP_BUFS = opts.get("expp_bufs", 13)
    TRI_ENGINE = opts.get("tri_engine", "vector")
    if True:
        NQC = L // 512
        # ---- phase B: q/k/v projections (transient weight+input pools) ----
        with tc.tile_pool(name="transient", bufs=1) as trans:
            xT_sb = trans.tile([128, 8, L], f32r, tag="xT")
            wq_sb = trans.tile([128, 8, WG], f32r, tag="wq")
            wk_sb = trans.tile([128, 8, WG], f32r, tag="wk")
            wv_sb = trans.tile([128, 8, WG], f32r, tag="wv")
            # DMAs in consumption order
            nc.sync.dma_start(out=wq_sb, in_=wq_d.rearrange("(c p) d -> p c d", p=128))
            xT_r = xT_d.rearrange("(c p) l -> p c l", p=128)
            xt_dmas = []
            for hc in range(8):
                xt_dmas.append(nc.sync.dma_start(out=xT_sb[:, hc, :], in_=xT_r[:, hc, :]))
            nc.sync.dma_start(out=wk_sb, in_=wk_d.rearrange("(c p) d -> p c d", p=128))
            nc.sync.dma_start(out=wv_sb, in_=wv_d.rearrange("(c p) d -> p c d", p=128))
            # phase-C constants: issue after the phase-B critical-path loads
            nc.sync.dma_start(out=wo_sb, in_=wo_d.rearrange("(c p) d -> p c d", p=128))
            nc.sync.dma_start(out=lnm_sb, in_=lnm_d.rearrange("(t p) -> p t", p=128))
            nc.sync.dma_start(out=tri_sb, in_=tri_d)
            # ones columns of v_aug (col 64 of each head block), via DMA'd
            # const (walrus rejects Memset with an f32r destination)
            ones_cols = v_aug.rearrange("p t (h j) -> p t h j", j=HD + 1)[
                :, :, :, HD : HD + 1
            ]
            nc.sync.dma_start(
                out=ones_cols,
                in_=onescol_d.rearrange("p (t h) -> p t h", h=NH)[:, :, :, None],
            )

            # qT / kT: stationary = weight chunk, moving = xT; hc-inner dense
            # accumulation groups. First matmul gated on xT chunk 3 so PE
            # starts at ~14us and then never starves behind the DMA stream
            # (q work per chunk 1.7us < 2.9us chunk DMA => chasing the stream
            # would stutter PE and drop it to the throttled pstate).
            from concourse.tile_rust import add_dep_helper

            with tc.tile_pool(name="qkv_ps", bufs=opts.get("qkv_bufs", 3), space="PSUM") as qkv_ps:
                first_mm = None

                def qk_proj(wt, dst, c2):
                    nonlocal first_mm
                    for qc in range(NQC):
                        ps = qkv_ps.tile([128, 512], f32, tag="qkps", name="qkps")
                        for hc in range(8):
                            mm = nc.tensor.matmul(
                                ps,
                                lhsT=wt[:, hc, 128 * c2 : 128 * c2 + 128],
                                rhs=xT_sb[:, hc, 512 * qc : 512 * qc + 512],
                                start=(hc == 0),
                                stop=(hc == 7),
                            )
                            if first_mm is None:
                                first_mm = mm
                                add_dep_helper(
                                    mm.ins,
                                    xt_dmas[GATE].ins,
                                    sync=True,
                                    reason="delay PE start to avoid stutter",
                                )
                        nc.vector.tensor_copy(
                            out=dst[:, c2, 512 * qc : 512 * qc + 512], in_=ps
                        )

                def v_proj():
                    for lt in range(NLT):
                        vps = qkv_ps.tile([128, 256], f32, tag="vps", name="vps")
                        for hc in range(8):
                            nc.tensor.matmul(
                                vps,
                                lhsT=xT_sb[:, hc, 128 * lt : 128 * lt + 128],
                                rhs=wv_sb[:, hc, :],
                                start=(hc == 0),
                                stop=(hc == 7),
                            )
                        dest = v_aug[:, lt, :].rearrange(
                            "p (h j) -> p h j", j=HD + 1
                        )[:, :, 0:HD]
                        src = vps.rearrange("p (h j) -> p h j", j=HD)
                        nc.vector.tensor_copy(out=dest, in_=src)

                # c2=0 projections + v first so heads 0/1 can start attention
                # while the c2=1 projections still run on PE
                qk_proj(wq_sb, qT, 0)
                qk_proj(wk_sb, kT, 0)
                v_proj()
                qk_proj(wq_sb, qT, 1)
                qk_proj(wk_sb, kT, 1)

        # ---- phase C: attention + wo ----
        from contextlib import ExitStack as _ES
        phc = _ES()
        s_ps = phc.enter_context(tc.tile_pool(name="s_ps", bufs=2, space="PSUM"))
        ctx_ps = phc.enter_context(tc.tile_pool(name="ctx_ps", bufs=1, space="PSUM"))
        wo_ps = phc.enter_context(tc.tile_pool(name="wo_ps", bufs=opts.get("wo_bufs", 2), space="PSUM"))
        expp = phc.enter_context(tc.tile_pool(name="expp", bufs=EXPP_BUFS))
        rp = phc.enter_context(tc.tile_pool(name="rp", bufs=opts.get("rp_bufs", 2)))
        ctxsbp = phc.enter_context(tc.tile_pool(name="ctxsbp", bufs=opts.get("ctxsb_bufs", 2)))
        outp = phc.enter_context(tc.tile_pool(name="outp", bufs=opts.get("outp_bufs", 8)))

        # Flat unit list (qh, h, kt) with a LOOKAHEAD: emit S+exp of the next
        # units before this unit's PV, so the next head's first exps are in
        # flight while the current head's diagonal tail drains (removes the
        # ACT idle bubble at each head boundary).
        LA = opts.get("lookahead", 12)
        units = []
        for qh in range(NQH):
            q0 = qh * QCH
            ktmax = (q0 + QCH - 1) // 128
            for h in range(NH):
                for kt in range(ktmax + 1):
                    o = max(0, 128 * kt - q0)
                    # chunk boundaries must be PSUM-bank-aligned (512 f32):
                    # matmul start=True clears at bank granularity.
                    chunks = []
                    n0 = o
                    while n0 < QCH:
                        n1 = min(QCH, (n0 // 512 + 1) * 512)
                        chunks.append((n0, n1, (q0 + n1 - 1) // 128))
                        n0 = n1
                    units.append((qh, h, kt, o, chunks))

        exp_tiles = {}

        def emit_se(i):
            qh, h, kt, o, chunks = units[i]
            q0 = qh * QCH
            p0 = HD * (h % 2)
            c2 = h // 2
            S = s_ps.tile([128, QCH], f32, tag="S", name="S")
            for (n0, n1, _) in chunks:
                nc.tensor.matmul(
                    S[:, n0:n1],
                    lhsT=kT[p0 : p0 + HD, c2, 128 * kt : 128 * kt + 128],
                    rhs=qT[p0 : p0 + HD, c2, q0 + n0 : q0 + n1],
                    start=True,
                    stop=True,
                )
            expS = expp.tile([128, QCH], f32r, tag="expS", name="expS")
            nc.scalar.activation(
                out=expS[:, o:QCH],
                in_=S[:, o:QCH],
                func=mybir.ActivationFunctionType.Exp,
                scale=INV_SQRT_D,
                bias=lnm_sb[:, kt : kt + 1],
            )
            if 128 * kt >= q0:  # diagonal block: causal triangle
                nc.vector.tensor_mul(
                    out=expS[:, o : o + 128],
                    in0=expS[:, o : o + 128],
                    in1=tri_sb,
                )
            exp_tiles[i] = expS

        def emit_wo(qh):
            q0 = qh * QCH
            for lt in range(QCH // 128):
                l0 = q0 + 128 * lt
                for n2 in range(2):
                    wps = wo_ps.tile([128, 512], f32, tag="wops", name="wops")
                    for cc in range(2):
                        nc.tensor.matmul(
                            wps,
                            lhsT=ctxT[cc][:, l0 : l0 + 128],
                            rhs=wo_sb[:, cc, 512 * n2 : 512 * n2 + 512],
                            start=(cc == 0),
                            stop=(cc == 1),
                        )
                    osb = outp.tile([128, 512], f32, tag="osb", name="osb")
                    nc.vector.tensor_copy(out=osb, in_=wps)
                    nc.sync.dma_start(
                        out=out_d[l0 : l0 + 128, 512 * n2 : 512 * n2 + 512], in_=osb
                    )

        se_cursor = 0
        ctx_t = None
        for i, (qh, h, kt, o, chunks) in enumerate(units):
            while se_cursor <= min(i + LA, len(units) - 1):
                emit_se(se_cursor)
                se_cursor += 1
            q0 = qh * QCH
            p0 = HD * (h % 2)
            c2 = h // 2
            if kt == 0:
                ctx_t = ctx_ps.tile([128, QCH], f32, tag="ctx", name="ctx")
            expS = exp_tiles.pop(i)
            for (n0, n1, kstop) in chunks:
                nc.tensor.matmul(
                    ctx_t[0 : HD + 1, n0:n1],
                    lhsT=v_aug[:, kt, (HD + 1) * h : (HD + 1) * (h + 1)],
                    rhs=expS[:, n0:n1],
                    start=(kt == 0),
                    stop=(kt == kstop),
                )
            ktmax = (q0 + QCH - 1) // 128
            if kt == ktmax:
                # normalize: ctxT[c2][p0:p0+64, q0:] = ctx[0:64] / rowsum
                ctx_sb = ctxsbp.tile([HD + 1, QCH], f32, tag="ctxsb", name="ctxsb")
                if opts.get("evac_engine", "vector") == "scalar":
                    nc.scalar.copy(out=ctx_sb, in_=ctx_t[0 : HD + 1, :])
                else:
                    nc.vector.tensor_copy(out=ctx_sb, in_=ctx_t[0 : HD + 1, :])
                r = rp.tile([1, QCH], f32, tag="r", name="r")
                nc.vector.reciprocal(r, ctx_sb[HD : HD + 1, :])
                r64 = rp.tile([HD, QCH], f32, tag="r64", name="r64")
                nc.gpsimd.partition_broadcast(r64, r)
                nc.vector.tensor_mul(
                    out=ctxT[c2][p0 : p0 + HD, q0 : q0 + QCH],
                    in0=ctx_sb[0:HD, :],
                    in1=r64,
                )
                if h == NH - 1:
                    emit_wo(qh)
        phc.close()


def _get_program(L=2048, QCH=1024):
    key = (L, QCH)
    if key not in _PROGRAM_CACHE:
        _PROGRAM_CACHE[key] = build_program(L, QCH)
    return _PROGRAM_CACHE[key]


def make_in_maps(x, am, wq, wk, wv, wo):
    """Shard full inputs into 8 per-core input maps."""
    B, L, _ = x.shape
    tri = np.triu(np.ones((128, 128), dtype=np.float32))
    in_maps = []
    for c in range(8):
        b, g = divmod(c, 4)
        cols = slice(WG * g, WG * (g + 1))
        m = am[b].astype(np.float32)
        lnm = np.where(m > 0, np.log(np.maximum(m, 1e-38)), -1e38).astype(np.float32)
        onescol = np.ones((128, (L // 128) * 4), dtype=np.float32)
        in_maps.append(
            {
                "onescol": onescol,
                "xT": np.ascontiguousarray(x[b].T),
                "wq": np.ascontiguousarray(wq[:, cols]),
                "wk": np.ascontiguousarray(wk[:, cols]),
                "wv": np.ascontiguousarray(wv[:, cols]),
                "wo": np.ascontiguousarray(wo[cols, :]),
                "lnm": lnm,
                "tri": tri,
            }
        )
    return in_maps


def kernel(**inputs):
    global LAST_RESULT
    x = np.asarray(inputs["input"], dtype=np.float32)
    am = np.asarray(inputs["attention_mask"], dtype=np.float32)
    wq = np.asarray(inputs["wq"], dtype=np.float32)
    wk = np.asarray(inputs["wk"], dtype=np.float32)
    wv = np.asarray(inputs["wv"], dtype=np.float32)
    wo = np.asarray(inputs["wo"], dtype=np.float32)
    B, L, _ = x.shape

    nc = _get_program(L=L, QCH=min(1024, L))
    in_maps = make_in_maps(x, am, wq, wk, wv, wo)
    trace = os.environ.get("KERNEL_TRACE", "0") == "1"
    res = run_bass_kernel_spmd(nc, in_maps, list(range(8)), trace=trace)
    LAST_RESULT = res

    out = np.zeros((B, L, H), dtype=np.float32)
    for b in range(B):
        for g in range(4):
            out[b] += res.results[4 * b + g]["out"]
    return out



# revision 45
# speedup vs baseline: 1.4844x; 1.4844x over previous
"""Decoder self-attention on 8 TRN2 NeuronCores.

Sharding: data-parallel over batch (2) x tensor-parallel over heads (4 groups
of 4 heads).  Core c handles batch c//4, heads 4*(c%4) .. 4*(c%4)+3.

v3 design (TimelineSim-guided, precision-validated):
  - q/k projections in SCALED fp8e4m3 DoubleRow (host sends 2*x and 32*w so
    fp8's subnormal floor is never hit; the 64x logit scale rides through the
    exp's scale argument).  Everything else on the ctx data path is bf16:
    fp8 noise (~2.4% rms) anywhere in the v/expS path translates ~1:1 into
    output error because ctx is a near-cancelling attention average, and the
    2e-2 budget only affords it on the q/k path (costs ~1.5e-2 via attention
    weights; numpy-emulated total 1.66e-2).
  - exp on the Act engine over PAIRED k-strips ([128, 2, 512] PSUM pair
    tiles, one call per pair) writing bf16 SBUF tiles consumed per-strip by
    bf16 PV matmuls.
  - causal masking: ONE gpsimd.affine_select per diagonal pair (iota affine
    in partition/parity/column covers both triangles and the odd strip's
    above-diagonal block).
  - pad mask folded into host inputs (x rows zeroed, v_aug ones-column = m).
  - normalize: DVE reciprocal of the PV rowsum row, gpsimd partition
    broadcast, DVE multiply into bf16 ctxT.  Final chunk's final head runs
    piecewise per l-tile, WO output DMA batched per l-tile.
"""

import os

import ml_dtypes
import numpy as np

import concourse.bass as bass
import concourse.tile as tile
from concourse import bacc, mybir
from concourse.bass_utils import run_bass_kernel_spmd

f32 = mybir.dt.float32
bf16 = mybir.dt.bfloat16
fp8 = mybir.dt.float8e4
DR = mybir.MatmulPerfMode.DoubleRow
EXP = mybir.ActivationFunctionType.Exp

H = 1024          # hidden dim
WG = 256          # weight-column group per core (4 heads x 64)
NH = 4            # heads per core
HD = 64           # head dim
QCH = 512         # q chunk
XS = 2.0          # host scale on fp8 x
WS = 32.0         # host scale on fp8 wq/wk

_PROGRAM_CACHE = {}
LAST_RESULT = None


def build_program(L=2048, QCH_=1024, opts=None):
    opts = dict(opts or {})
    qk8 = opts.get("qk8", True)
    NLT = L // 128
    NQH = L // QCH
    nc = bacc.Bacc("TRN2", target_bir_lowering=False, debug=False)

    x8_d = nc.dram_tensor("xT8", [H, L], fp8, kind="ExternalInput").ap()
    x8l_d = nc.dram_tensor("xT8l", [H, L], fp8, kind="ExternalInput").ap()
    w8_d = nc.dram_tensor("w8", [H, 4 * WG], fp8, kind="ExternalInput").ap()
    wo_d = nc.dram_tensor("wo16", [WG, H], bf16, kind="ExternalInput").ap()
    mcol_d = nc.dram_tensor("mcol", [128, NLT * NH], bf16, kind="ExternalInput").ap()
    out_d = nc.dram_tensor("out", [L, H], bf16, kind="ExternalOutput").ap()

    from contextlib import ExitStack

    with ExitStack() as ctx:
        tc = ctx.enter_context(tile.TileContext(nc))
        persist = ctx.enter_context(tc.tile_pool(name="persist", bufs=1))
        x8_sb = persist.tile([128, 8, L], fp8, tag="x8")
        x8l_sb = persist.tile([128, 8, L], fp8, tag="x8l")
        w8_sb = persist.tile([128, 8, 4 * WG], fp8, tag="w8")
        wo_sb = persist.tile([128, 2, H], bf16, tag="wo")
        qT = persist.tile([128, 2, L], bf16, tag="qT")
        kT = persist.tile([128, 2, L], bf16, tag="kT")
        v_aug = persist.tile([128, NLT, NH * (HD + 1)], bf16, tag="vaug")
        ctxT = persist.tile([128, 2, L], bf16, tag="ctxT")

        # ---- DMAs (sync/SP queue, consumption order) ----
        # w8 column order: [wq_c2=0|wk_c2=0 (0:256) | wvh (256:512) |
        #                   wvl (512:768) | wq_c2=1|wk_c2=1 (768:1024)]
        w8_r = w8_d.rearrange("(c p) d -> p c d", p=128)
        x8_r = x8_d.rearrange("(c p) l -> p c l", p=128)
        x8l_r = x8l_d.rearrange("(c p) l -> p c l", p=128)
        ones_cols = v_aug.rearrange("p t (h j) -> p t h j", j=HD + 1)[
            :, :, :, HD : HD + 1
        ]
        mcol_r = mcol_d.rearrange("p (t h) -> p t h", h=NH)
        nc.sync.dma_start(out=w8_sb[:, :, 0:256], in_=w8_r[:, :, 0:256])
        for lc in range(NQH):
            nc.sync.dma_start(
                out=x8_sb[:, :, QCH * lc : QCH * lc + QCH],
                in_=x8_r[:, :, QCH * lc : QCH * lc + QCH],
            )
            nc.sync.dma_start(
                out=x8l_sb[:, :, QCH * lc : QCH * lc + QCH],
                in_=x8l_r[:, :, QCH * lc : QCH * lc + QCH],
            )
            if lc == 0:
                nc.sync.dma_start(
                    out=w8_sb[:, :, 256:768], in_=w8_r[:, :, 256:768]
                )
            if lc == 0:
                nc.sync.dma_start(
                    out=ones_cols[:, 0:4], in_=mcol_r[:, 0:4, :, None]
                )
                nc.sync.dma_start(
                    out=w8_sb[:, :, 768:1024], in_=w8_r[:, :, 768:1024]
                )
            if lc == 1:
                nc.sync.dma_start(
                    out=ones_cols[:, 4:NLT], in_=mcol_r[:, 4:NLT, :, None]
                )
        nc.sync.dma_start(out=wo_sb, in_=wo_d.rearrange("(c p) d -> p c d", p=128))

        # ---- pools (PSUM: spair 2x2 + ctx 2x1 + scratch 2x1 = 8 banks) ----
        s_ps = ctx.enter_context(
            tc.tile_pool(name="s_ps", bufs=opts.get("s_bufs", 2), space="PSUM")
        )
        ctx_ps = ctx.enter_context(
            tc.tile_pool(name="ctx_ps", bufs=opts.get("ctx_bufs", 2), space="PSUM")
        )
        scratch = ctx.enter_context(
            tc.tile_pool(name="scratch", bufs=opts.get("scr_bufs", 2), space="PSUM")
        )
        expp = ctx.enter_context(
            tc.tile_pool(name="expp", bufs=opts.get("expp_bufs", 10))
        )
        rp = ctx.enter_context(tc.tile_pool(name="rp", bufs=2))
        rbp = ctx.enter_context(tc.tile_pool(name="rbp", bufs=2))
        outp = ctx.enter_context(tc.tile_pool(name="outp", bufs=opts.get("outp_bufs", 4)))

        fill0 = nc.gpsimd.to_reg(0.0)

        # PE p-state warmup: ~3us of throwaway matmuls so the real
        # projections start at full clock.
        dummy = persist.tile([128, 2, 128], fp8, tag="dummy")
        nc.gpsimd.memset(dummy, 0.0)
        wps = scratch.tile([128, QCH], f32, tag="scr", name="warm")
        for _ in range(opts.get("warm", 60)):
            nc.tensor.matmul(
                wps[:, 0:128], lhsT=dummy, rhs=dummy,
                start=True, stop=True, perf_mode=DR,
            )

        def qk_proj(c2, lc, which=(0, 1)):
            for wi, dst in ((0, qT), (1, kT)):
                if wi not in which:
                    continue
                ps = scratch.tile([128, QCH], f32, tag="scr", name="qkps")
                w0 = (768 if c2 else 0) + 128 * wi
                for p in range(4):
                    for n in range(2):
                        nc.tensor.matmul(
                            ps[:, 256 * n : 256 * n + 256],
                            lhsT=w8_sb[:, 2 * p : 2 * p + 2, w0 : w0 + 128],
                            rhs=x8_sb[
                                :, 2 * p : 2 * p + 2,
                                QCH * lc + 256 * n : QCH * lc + 256 * n + 256,
                            ],
                            start=(p == 0 and n == 0),
                            stop=(p == 3 and n == 1),
                            perf_mode=DR,
                        )
                if lc == 0 and c2 == 0:
                    # evac the 256:512 half first — the first attention pair
                    # (j=1) reads exactly those columns
                    nc.vector.tensor_copy(
                        out=dst[:, c2, 256:512], in_=ps[:, 256:512]
                    )
                    nc.vector.tensor_copy(out=dst[:, c2, 0:256], in_=ps[:, 0:256])
                else:
                    nc.vector.tensor_copy(
                        out=dst[:, c2, QCH * lc : QCH * lc + QCH], in_=ps
                    )

        def v_proj(lc):
            # v scaled by XS*WS; hi-lo fp8: xh*wh + xl*wh + xh*wl
            terms = [(x8_sb, 256), (x8l_sb, 256), (x8_sb, 512)]
            for lt in range(4 * lc, 4 * lc + 4):
                ps = scratch.tile([128, QCH], f32, tag="scr", name="vps")
                for ti, (xs, wv0) in enumerate(terms):
                    for p in range(4):
                        nc.tensor.matmul(
                            ps[:, 0:WG],
                            lhsT=xs[:, 2 * p : 2 * p + 2, 128 * lt : 128 * lt + 128],
                            rhs=w8_sb[:, 2 * p : 2 * p + 2, wv0 : wv0 + WG],
                            start=(ti == 0 and p == 0),
                            stop=(ti == 2 and p == 3),
                            perf_mode=DR,
                        )
                dest = v_aug[:, lt, :].rearrange("p (h j) -> p h j", j=HD + 1)[
                    :, :, 0:HD
                ]
                nc.vector.tensor_copy(
                    out=dest, in_=ps[:, 0:WG].rearrange("p (h j) -> p h j", j=HD)
                )

        # ---- attention units ----
        # Within a head, process the two diagonal pairs FIRST so their
        # Pool-engine affine masks are off the end-of-head critical path.
        units = []
        for qh in range(NQH):
            jorder = [2 * qh + 1, 2 * qh] + list(range(2 * qh))
            for h in range(NH):
                for j in jorder:
                    units.append((qh, h, j))
        N = len(units)

        exp_tiles = {}
        ctx_cur = {}
        SDIV = 32.0 * (XS * WS) ** 2 if qk8 else 32.0

        def emit_se(i):
            qh, h, j = units[i]
            if qh == 0 and j == 2 * qh + 1:
                if h == 0:
                    qk_proj(0, 0)
                    v_proj(0)
                elif h == 1:
                    qk_proj(1, 0)
                    qk_proj(0, 1, which=(0,))
                elif h == 2:
                    qk_proj(0, 1, which=(1,))
                    v_proj(1)
                else:
                    qk_proj(1, 1)
            q0 = QCH * qh
            p0 = HD * (h % 2)
            c2 = h // 2
            o = 256 if j == 2 * qh + 1 else 0
            s = s_ps.tile([128, 2, QCH], f32, tag="spair", name="spair")
            for parity in range(2):
                kt = 2 * j + parity
                os_ = max(0, 128 * kt - q0)
                nc.tensor.matmul(
                    s[:, parity, os_:QCH],
                    lhsT=kT[p0 : p0 + HD, c2, 128 * kt : 128 * kt + 128],
                    rhs=qT[p0 : p0 + HD, c2, q0 + os_ : q0 + QCH],
                    start=True,
                    stop=True,
                )
            ep = expp.tile([128, 2, QCH], bf16, tag="ep", name="ep")
            nc.scalar.activation(
                out=ep[:, :, o:QCH], in_=s[:, :, o:QCH], func=EXP, scale=1.0 / SDIV
            )
            if j >= 2 * qh:  # diagonal pair: causal triangle via affine iota
                nc.gpsimd.affine_select(
                    out=ep[:, :, o : o + 256],
                    in_=ep[:, :, o : o + 256],
                    pattern=[[-128, 2], [1, 256]],
                    compare_op=mybir.AluOpType.is_ge,
                    fill=fill0,
                    base=0,
                    channel_multiplier=-1,
                )
            exp_tiles[i] = ep

        def wo_lt(l0, tail):
            osb = outp.tile([128, H], bf16, tag="osb", name="osb")
            for n2 in range(2):
                w = scratch.tile([128, 512], f32, tag="scr", name="wops")
                for cc in range(2):
                    nc.tensor.matmul(
                        w,
                        lhsT=ctxT[:, cc, l0 : l0 + 128],
                        rhs=wo_sb[:, cc, 512 * n2 : 512 * n2 + 512],
                        start=(cc == 0),
                        stop=(cc == 1),
                    )
                if tail:
                    nc.scalar.copy(
                        out=osb[:, 512 * n2 : 512 * n2 + 512], in_=w
                    )  # Act idle in tail; keep DVE free for normalize pieces
                else:
                    nc.vector.tensor_copy(
                        out=osb[:, 512 * n2 : 512 * n2 + 512], in_=w
                    )
            nc.sync.dma_start(out=out_d[l0 : l0 + 128, :], in_=osb)

        def emit_pv(i):
            qh, h, j = units[i]
            q0 = QCH * qh
            p0 = HD * (h % 2)
            c2 = h // 2
            jfirst = 2 * qh + 1
            jlast = 2 * qh - 1 if qh > 0 else 0
            if j == jfirst and 0 < qh < NQH - 1:
                # stage next chunk's projections in pieces so the PE absorbs
                # them without starving the Act exp stream
                if h == 0:
                    qk_proj(0, qh + 1, which=(0,))
                elif h == 1:
                    qk_proj(0, qh + 1, which=(1,))
                elif h == 2:
                    v_proj(qh + 1)
                else:
                    qk_proj(1, qh + 1)
            if j == jfirst:
                ctx_cur[h] = ctx_ps.tile([128, QCH], f32, tag="ctx", name="ctx")
            ctx_t = ctx_cur[h]
            ep = exp_tiles.pop(i)
            for parity in range(2):
                kt = 2 * j + parity
                os_ = max(0, 128 * kt - q0)
                nc.tensor.matmul(
                    ctx_t[0 : HD + 1, os_:QCH],
                    lhsT=v_aug[:, kt, (HD + 1) * h : (HD + 1) * (h + 1)],
                    rhs=ep[:, parity, os_:QCH],
                    start=(j == jfirst and parity == 0),
                    stop=(j == jlast and parity == 1),
                )
            if j != jlast:
                return
            tailpiece = qh == NQH - 1 and h == NH - 1
            if not tailpiece:
                r = rp.tile([1, QCH], f32, tag="r", name="r")
                nc.vector.reciprocal(r, ctx_t[HD : HD + 1, :])
                r64 = rbp.tile([HD, QCH], f32, tag="r64", name="r64")
                nc.gpsimd.partition_broadcast(r64, r)
                nc.vector.tensor_mul(
                    out=ctxT[p0 : p0 + HD, c2, q0 : q0 + QCH],
                    in0=ctx_t[0:HD, :],
                    in1=r64,
                )
                if h == NH - 1:
                    for lt in range(4):
                        wo_lt(q0 + 128 * lt, tail=False)
            else:
                # final chunk's final head: piecewise normalize + WO to
                # minimize the serial tail
                for lt in range(4):
                    sl = slice(128 * lt, 128 * lt + 128)
                    rr = rp.tile([1, 128], f32, tag="rpc", name="rr")
                    nc.vector.reciprocal(rr, ctx_t[HD : HD + 1, sl])
                    r64 = rbp.tile([HD, 128], f32, tag="r64pc", name="r64pc")
                    nc.gpsimd.partition_broadcast(r64, rr)
                    nc.vector.tensor_mul(
                        out=ctxT[p0 : p0 + HD, c2, q0 + 128 * lt : q0 + 128 * lt + 128],
                        in0=ctx_t[0:HD, sl],
                        in1=r64,
                    )
                    wo_lt(q0 + 128 * lt, tail=True)

        LA = opts.get("lookahead", 2)
        se_cursor = 0
        for i in range(N):
            while se_cursor <= min(i + LA, N - 1):
                emit_se(se_cursor)
                se_cursor += 1
            emit_pv(i)

    nc.compile()
    return nc


def _get_program(L=2048, QCH_=1024):
    key = (L, QCH_)
    if key not in _PROGRAM_CACHE:
        _PROGRAM_CACHE[key] = build_program(L, QCH_)
    return _PROGRAM_CACHE[key]


def make_in_maps(x, am, wq, wk, wv, wo):
    B, L, _ = x.shape
    NLT = L // 128
    e4 = ml_dtypes.float8_e4m3
    bf = ml_dtypes.bfloat16
    in_maps = []
    for c in range(8):
        b, g = divmod(c, 4)
        cols = slice(WG * g, WG * (g + 1))
        m = am[b].astype(np.float32)
        xm = (x[b] * m[:, None]).astype(np.float32)
        wq_c, wk_c = wq[:, cols], wk[:, cols]
        # v weights: scaled hi-lo fp8 pair (wvh + wvl == 32*wv to ~bf16 acc)
        wvs = (WS * wv[:, cols]).astype(np.float32)
        wvh = wvs.astype(e4)
        wvl = (wvs - wvh.astype(np.float32)).astype(e4)
        w8 = np.concatenate(
            [
                (WS * wq_c[:, 0:128]).astype(e4),
                (WS * wk_c[:, 0:128]).astype(e4),
                wvh,
                wvl,
                (WS * wq_c[:, 128:256]).astype(e4),
                (WS * wk_c[:, 128:256]).astype(e4),
            ],
            axis=1,
        )
        # ones column carries XS*WS*m so the reciprocal absorbs the v scale
        mcol = np.repeat(
            (XS * WS * m).reshape(NLT, 128).T[:, :, None], NH, axis=2
        ).reshape(128, NLT * NH)
        xT = np.ascontiguousarray(xm.T)
        x8 = (XS * xT).astype(e4)
        x8l = (XS * xT - x8.astype(np.float32)).astype(e4)
        in_maps.append(
            {
                "xT8": x8,
                "xT8l": x8l,
                "w8": np.ascontiguousarray(w8),
                "wo16": np.ascontiguousarray(wo[cols, :]).astype(bf),
                "mcol": np.ascontiguousarray(mcol).astype(bf),
            }
        )
    return in_maps


def kernel(**inputs):
    global LAST_RESULT
    x = np.asarray(inputs["input"], dtype=np.float32)
    am = np.asarray(inputs["attention_mask"], dtype=np.float32)
    wq = np.asarray(inputs["wq"], dtype=np.float32)
    wk = np.asarray(inputs["wk"], dtype=np.float32)
    wv = np.asarray(inputs["wv"], dtype=np.float32)
    wo = np.asarray(inputs["wo"], dtype=np.float32)
    B, L, _ = x.shape

    nc = _get_program(L=L, QCH_=min(1024, L))
    in_maps = make_in_maps(x, am, wq, wk, wv, wo)
    trace = os.environ.get("KERNEL_TRACE", "0") == "1"
    res = run_bass_kernel_spmd(nc, in_maps, list(range(8)), trace=trace)
    LAST_RESULT = res

    out = np.zeros((B, L, 1024), dtype=np.float32)
    for b in range(B):
        for g in range(4):
            out[b] += res.results[4 * b + g]["out"].astype(np.float32)
    return out


# revision 55
# speedup vs baseline: 1.6123x; 1.0862x over previous
"""Decoder self-attention on 8 TRN2 NeuronCores.

Sharding: data-parallel over batch (2) x tensor-parallel over heads (4 groups
of 4 heads).  Core c handles batch c//4, heads 4*(c%4) .. 4*(c%4)+3.

v3 design (TimelineSim-guided, precision-validated):
  - q/k projections in SCALED fp8e4m3 DoubleRow (host sends 2*x and 32*w so
    fp8's subnormal floor is never hit; the 64x logit scale rides through the
    exp's scale argument).  Everything else on the ctx data path is bf16:
    fp8 noise (~2.4% rms) anywhere in the v/expS path translates ~1:1 into
    output error because ctx is a near-cancelling attention average, and the
    2e-2 budget only affords it on the q/k path (costs ~1.5e-2 via attention
    weights; numpy-emulated total 1.66e-2).
  - exp on the Act engine over PAIRED k-strips ([128, 2, 512] PSUM pair
    tiles, one call per pair) writing bf16 SBUF tiles consumed per-strip by
    bf16 PV matmuls.
  - causal masking: ONE gpsimd.affine_select per diagonal pair (iota affine
    in partition/parity/column covers both triangles and the odd strip's
    above-diagonal block).
  - pad mask folded into host inputs (x rows zeroed, v_aug ones-column = m).
  - normalize: DVE reciprocal of the PV rowsum row, gpsimd partition
    broadcast, DVE multiply into bf16 ctxT.  Final chunk's final head runs
    piecewise per l-tile, WO output DMA batched per l-tile.
"""

import os

import ml_dtypes
import numpy as np

import concourse.bass as bass
import concourse.tile as tile
from concourse import bacc, mybir
from concourse.bass_utils import run_bass_kernel_spmd

f32 = mybir.dt.float32
bf16 = mybir.dt.bfloat16
fp8 = mybir.dt.float8e4
DR = mybir.MatmulPerfMode.DoubleRow
EXP = mybir.ActivationFunctionType.Exp

H = 1024          # hidden dim
WG = 256          # weight-column group per core (4 heads x 64)
NH = 4            # heads per core
HD = 64           # head dim
QCH = 512         # q chunk
XS = 2.0          # host scale on fp8 x
WS = 32.0         # host scale on fp8 wq/wk

_PROGRAM_CACHE = {}
LAST_RESULT = None


def build_program(L=2048, QCH_=1024, opts=None):
    opts = dict(opts or {})
    qk8 = opts.get("qk8", True)
    NLT = L // 128
    NQH = L // QCH
    nc = bacc.Bacc("TRN2", target_bir_lowering=False, debug=False)

    x8_d = nc.dram_tensor("xT8", [H, L], fp8, kind="ExternalInput").ap()
    x8l_d = nc.dram_tensor("xT8l", [H, L], fp8, kind="ExternalInput").ap()
    w8_d = nc.dram_tensor("w8", [H, 4 * WG], fp8, kind="ExternalInput").ap()
    wo_d = nc.dram_tensor("wo16", [WG, H], bf16, kind="ExternalInput").ap()
    mcol_d = nc.dram_tensor("mcol", [128, NLT * NH], bf16, kind="ExternalInput").ap()
    out_d = nc.dram_tensor("out", [L, H], bf16, kind="ExternalOutput").ap()

    from contextlib import ExitStack

    with ExitStack() as ctx:
        tc = ctx.enter_context(tile.TileContext(nc))
        persist = ctx.enter_context(tc.tile_pool(name="persist", bufs=1))
        x8_sb = persist.tile([128, 8, L], fp8, tag="x8")
        x8l_sb = persist.tile([128, 8, L], fp8, tag="x8l")
        w8_sb = persist.tile([128, 8, 4 * WG], fp8, tag="w8")
        wo_sb = persist.tile([128, 2, H], bf16, tag="wo")
        qT = persist.tile([128, 2, L], bf16, tag="qT")
        kT = persist.tile([128, 2, L], bf16, tag="kT")
        v_aug = persist.tile([128, NLT, NH * (HD + 1)], bf16, tag="vaug")
        ctxT = persist.tile([128, 2, L], bf16, tag="ctxT")

        # ---- DMAs (sync/SP queue, consumption order) ----
        # w8 column order: [wq_c2=0|wk_c2=0 (0:256) | wvh (256:512) |
        #                   wvl (512:768) | wq_c2=1|wk_c2=1 (768:1024)]
        w8_r = w8_d.rearrange("(c p) d -> p c d", p=128)
        x8_r = x8_d.rearrange("(c p) l -> p c l", p=128)
        x8l_r = x8l_d.rearrange("(c p) l -> p c l", p=128)
        ones_cols = v_aug.rearrange("p t (h j) -> p t h j", j=HD + 1)[
            :, :, :, HD : HD + 1
        ]
        mcol_r = mcol_d.rearrange("p (t h) -> p t h", h=NH)
        nc.sync.dma_start(out=w8_sb[:, :, 0:256], in_=w8_r[:, :, 0:256])
        for lc in range(NQH):
            if lc == 0:
                # split the first chunk so the first projection half-chunk
                # starts one DMA earlier
                nc.sync.dma_start(out=x8_sb[:, :, 0:256], in_=x8_r[:, :, 0:256])
                nc.sync.dma_start(out=x8_sb[:, :, 256:512], in_=x8_r[:, :, 256:512])
            else:
                nc.sync.dma_start(
                    out=x8_sb[:, :, QCH * lc : QCH * lc + QCH],
                    in_=x8_r[:, :, QCH * lc : QCH * lc + QCH],
                )
            nc.sync.dma_start(
                out=x8l_sb[:, :, QCH * lc : QCH * lc + QCH],
                in_=x8l_r[:, :, QCH * lc : QCH * lc + QCH],
            )
            if lc == 0:
                nc.sync.dma_start(
                    out=w8_sb[:, :, 256:768], in_=w8_r[:, :, 256:768]
                )
            if lc == 0:
                nc.sync.dma_start(
                    out=ones_cols[:, 0:4], in_=mcol_r[:, 0:4, :, None]
                )
                nc.sync.dma_start(
                    out=w8_sb[:, :, 768:1024], in_=w8_r[:, :, 768:1024]
                )
            if lc == 1:
                nc.sync.dma_start(
                    out=ones_cols[:, 4:NLT], in_=mcol_r[:, 4:NLT, :, None]
                )
        nc.sync.dma_start(out=wo_sb, in_=wo_d.rearrange("(c p) d -> p c d", p=128))

        # ---- pools (PSUM: spair 2x2 + ctx 2x1 + scratch 2x1 = 8 banks) ----
        s_ps = ctx.enter_context(
            tc.tile_pool(name="s_ps", bufs=opts.get("s_bufs", 2), space="PSUM")
        )
        ctx_ps = ctx.enter_context(
            tc.tile_pool(name="ctx_ps", bufs=opts.get("ctx_bufs", 2), space="PSUM")
        )
        scratch = ctx.enter_context(
            tc.tile_pool(name="scratch", bufs=opts.get("scr_bufs", 2), space="PSUM")
        )
        expp = ctx.enter_context(
            tc.tile_pool(name="expp", bufs=opts.get("expp_bufs", 14))
        )
        rp = ctx.enter_context(tc.tile_pool(name="rp", bufs=2))
        rbp = ctx.enter_context(tc.tile_pool(name="rbp", bufs=2))
        outp = ctx.enter_context(tc.tile_pool(name="outp", bufs=opts.get("outp_bufs", 4)))

        fill0 = nc.gpsimd.to_reg(0.0)

        # PE p-state warmup: ~3us of throwaway matmuls so the real
        # projections start at full clock.
        dummy = persist.tile([128, 2, 128], fp8, tag="dummy")
        nc.gpsimd.memset(dummy, 0.0)
        wps = scratch.tile([128, QCH], f32, tag="scr", name="warm")
        for _ in range(opts.get("warm", 60)):
            nc.tensor.matmul(
                wps[:, 0:128], lhsT=dummy, rhs=dummy,
                start=True, stop=True, perf_mode=DR,
            )

        def qk_proj(c2, lc, which=(0, 1)):
            for wi, dst in ((0, qT), (1, kT)):
                if wi not in which:
                    continue
                ps = scratch.tile([128, QCH], f32, tag="scr", name="qkps")
                w0 = (768 if c2 else 0) + 128 * wi
                for p in range(4):
                    for n in range(2):
                        nc.tensor.matmul(
                            ps[:, 256 * n : 256 * n + 256],
                            lhsT=w8_sb[:, 2 * p : 2 * p + 2, w0 : w0 + 128],
                            rhs=x8_sb[
                                :, 2 * p : 2 * p + 2,
                                QCH * lc + 256 * n : QCH * lc + 256 * n + 256,
                            ],
                            start=(p == 0 and n == 0),
                            stop=(p == 3 and n == 1),
                            perf_mode=DR,
                        )
                if lc == 0 and c2 == 0:
                    # evac the 256:512 half first — the first attention pair
                    # (j=1) reads exactly those columns
                    nc.vector.tensor_copy(
                        out=dst[:, c2, 256:512], in_=ps[:, 256:512]
                    )
                    nc.vector.tensor_copy(out=dst[:, c2, 0:256], in_=ps[:, 0:256])
                else:
                    nc.vector.tensor_copy(
                        out=dst[:, c2, QCH * lc : QCH * lc + QCH], in_=ps
                    )

        def v_proj(lc):
            # v scaled by XS*WS; hi-lo fp8: xh*wh + xl*wh + xh*wl
            terms = [(x8_sb, 256), (x8l_sb, 256), (x8_sb, 512)]
            for lt in range(4 * lc, 4 * lc + 4):
                ps = scratch.tile([128, QCH], f32, tag="scr", name="vps")
                for ti, (xs, wv0) in enumerate(terms):
                    for p in range(4):
                        nc.tensor.matmul(
                            ps[:, 0:WG],
                            lhsT=xs[:, 2 * p : 2 * p + 2, 128 * lt : 128 * lt + 128],
                            rhs=w8_sb[:, 2 * p : 2 * p + 2, wv0 : wv0 + WG],
                            start=(ti == 0 and p == 0),
                            stop=(ti == 2 and p == 3),
                            perf_mode=DR,
                        )
                dest = v_aug[:, lt, :].rearrange("p (h j) -> p h j", j=HD + 1)[
                    :, :, 0:HD
                ]
                nc.vector.tensor_copy(
                    out=dest, in_=ps[:, 0:WG].rearrange("p (h j) -> p h j", j=HD)
                )

        # ---- attention units ----
        # Within a head, process the two diagonal pairs FIRST so their
        # Pool-engine affine masks are off the end-of-head critical path.
        units = []
        for qh in range(NQH):
            jorder = [2 * qh + 1, 2 * qh] + list(range(2 * qh))
            for h in range(NH):
                for j in jorder:
                    units.append((qh, h, j))
        N = len(units)

        exp_tiles = {}
        ctx_cur = {}
        pending_wo = []
        SDIV = 32.0 * (XS * WS) ** 2 if qk8 else 32.0

        def emit_se(i):
            qh, h, j = units[i]
            if qh == 0 and j == 2 * qh + 1:
                if h == 0:
                    qk_proj(0, 0)
                    v_proj(0)
                elif h == 1:
                    qk_proj(1, 0)
                    qk_proj(0, 1, which=(0,))
                elif h == 2:
                    qk_proj(0, 1, which=(1,))
                    v_proj(1)
                else:
                    qk_proj(1, 1)
            q0 = QCH * qh
            p0 = HD * (h % 2)
            c2 = h // 2
            o = 256 if j == 2 * qh + 1 else 0
            s = s_ps.tile([128, 2, QCH], f32, tag="spair", name="spair")
            for parity in range(2):
                kt = 2 * j + parity
                os_ = max(0, 128 * kt - q0)
                nc.tensor.matmul(
                    s[:, parity, os_:QCH],
                    lhsT=kT[p0 : p0 + HD, c2, 128 * kt : 128 * kt + 128],
                    rhs=qT[p0 : p0 + HD, c2, q0 + os_ : q0 + QCH],
                    start=True,
                    stop=True,
                )
            ep = expp.tile([128, 2, QCH], bf16, tag="ep", name="ep")
            nc.scalar.activation(
                out=ep[:, :, o:QCH], in_=s[:, :, o:QCH], func=EXP, scale=1.0 / SDIV
            )
            if j >= 2 * qh:  # diagonal pair: causal triangle via affine iota
                nc.gpsimd.affine_select(
                    out=ep[:, :, o : o + 256],
                    in_=ep[:, :, o : o + 256],
                    pattern=[[-128, 2], [1, 256]],
                    compare_op=mybir.AluOpType.is_ge,
                    fill=fill0,
                    base=0,
                    channel_multiplier=-1,
                )
            exp_tiles[i] = ep

        def wo_lt(l0, tail):
            osb = outp.tile([128, H], bf16, tag="osb", name="osb")
            for n2 in range(2):
                w = scratch.tile([128, 512], f32, tag="scr", name="wops")
                for cc in range(2):
                    nc.tensor.matmul(
                        w,
                        lhsT=ctxT[:, cc, l0 : l0 + 128],
                        rhs=wo_sb[:, cc, 512 * n2 : 512 * n2 + 512],
                        start=(cc == 0),
                        stop=(cc == 1),
                    )
                if tail:
                    nc.scalar.copy(
                        out=osb[:, 512 * n2 : 512 * n2 + 512], in_=w
                    )  # Act idle in tail; keep DVE free for normalize pieces
                else:
                    nc.vector.tensor_copy(
                        out=osb[:, 512 * n2 : 512 * n2 + 512], in_=w
                    )
            nc.sync.dma_start(out=out_d[l0 : l0 + 128, :], in_=osb)

        def emit_pv(i):
            qh, h, j = units[i]
            q0 = QCH * qh
            p0 = HD * (h % 2)
            c2 = h // 2
            jfirst = 2 * qh + 1
            jlast = 2 * qh - 1 if qh > 0 else 0
            if j == jfirst and 0 < qh < NQH - 1:
                # stage next chunk's projections in pieces so the PE absorbs
                # them without starving the Act exp stream
                if h == 0:
                    qk_proj(0, qh + 1, which=(0,))
                elif h == 1:
                    qk_proj(0, qh + 1, which=(1,))
                elif h == 2:
                    v_proj(qh + 1)
                else:
                    qk_proj(1, qh + 1)
            if pending_wo:
                wo_lt(pending_wo.pop(0), tail=False)
            if j == jfirst:
                ctx_cur[h] = ctx_ps.tile([128, QCH], f32, tag="ctx", name="ctx")
            ctx_t = ctx_cur[h]
            ep = exp_tiles.pop(i)
            for parity in range(2):
                kt = 2 * j + parity
                os_ = max(0, 128 * kt - q0)
                nc.tensor.matmul(
                    ctx_t[0 : HD + 1, os_:QCH],
                    lhsT=v_aug[:, kt, (HD + 1) * h : (HD + 1) * (h + 1)],
                    rhs=ep[:, parity, os_:QCH],
                    start=(j == jfirst and parity == 0),
                    stop=(j == jlast and parity == 1),
                )
            if j != jlast:
                return
            tailpiece = qh == NQH - 1 and h == NH - 1
            if not tailpiece:
                r = rp.tile([1, QCH], f32, tag="r", name="r")
                nc.vector.reciprocal(r, ctx_t[HD : HD + 1, :])
                r64 = rbp.tile([HD, QCH], f32, tag="r64", name="r64")
                nc.gpsimd.partition_broadcast(r64, r)
                nc.vector.tensor_mul(
                    out=ctxT[p0 : p0 + HD, c2, q0 : q0 + QCH],
                    in0=ctx_t[0:HD, :],
                    in1=r64,
                )
                if h == NH - 1:
                    # spread this chunk's WO over the next chunk's pv units
                    pending_wo.extend(q0 + 128 * lt for lt in range(4))
            else:
                # final chunk's final head: piecewise normalize + WO to
                # minimize the serial tail
                for lt in range(4):
                    sl = slice(128 * lt, 128 * lt + 128)
                    rr = rp.tile([1, 128], f32, tag="rpc", name="rr")
                    nc.vector.reciprocal(rr, ctx_t[HD : HD + 1, sl])
                    r64 = rbp.tile([HD, 128], f32, tag="r64pc", name="r64pc")
                    nc.gpsimd.partition_broadcast(r64, rr)
                    nc.vector.tensor_mul(
                        out=ctxT[p0 : p0 + HD, c2, q0 + 128 * lt : q0 + 128 * lt + 128],
                        in0=ctx_t[0:HD, sl],
                        in1=r64,
                    )
                    wo_lt(q0 + 128 * lt, tail=True)

        LA = opts.get("lookahead", 12)
        se_cursor = 0
        for i in range(N):
            while se_cursor <= min(i + LA, N - 1):
                emit_se(se_cursor)
                se_cursor += 1
            emit_pv(i)

    nc.compile()
    return nc


def _get_program(L=2048, QCH_=1024):
    key = (L, QCH_)
    if key not in _PROGRAM_CACHE:
        _PROGRAM_CACHE[key] = build_program(L, QCH_)
    return _PROGRAM_CACHE[key]


def make_in_maps(x, am, wq, wk, wv, wo):
    B, L, _ = x.shape
    NLT = L // 128
    e4 = ml_dtypes.float8_e4m3
    bf = ml_dtypes.bfloat16
    in_maps = []
    for c in range(8):
        b, g = divmod(c, 4)
        cols = slice(WG * g, WG * (g + 1))
        m = am[b].astype(np.float32)
        xm = (x[b] * m[:, None]).astype(np.float32)
        wq_c, wk_c = wq[:, cols], wk[:, cols]
        # v weights: scaled hi-lo fp8 pair (wvh + wvl == 32*wv to ~bf16 acc)
        wvs = (WS * wv[:, cols]).astype(np.float32)
        wvh = wvs.astype(e4)
        wvl = (wvs - wvh.astype(np.float32)).astype(e4)
        w8 = np.concatenate(
            [
                (WS * wq_c[:, 0:128]).astype(e4),
                (WS * wk_c[:, 0:128]).astype(e4),
                wvh,
                wvl,
                (WS * wq_c[:, 128:256]).astype(e4),
                (WS * wk_c[:, 128:256]).astype(e4),
            ],
            axis=1,
        )
        # ones column carries XS*WS*m so the reciprocal absorbs the v scale
        mcol = np.repeat(
            (XS * WS * m).reshape(NLT, 128).T[:, :, None], NH, axis=2
        ).reshape(128, NLT * NH)
        xT = np.ascontiguousarray(xm.T)
        x8 = (XS * xT).astype(e4)
        x8l = (XS * xT - x8.astype(np.float32)).astype(e4)
        in_maps.append(
            {
                "xT8": x8,
                "xT8l": x8l,
                "w8": np.ascontiguousarray(w8),
                "wo16": np.ascontiguousarray(wo[cols, :]).astype(bf),
                "mcol": np.ascontiguousarray(mcol).astype(bf),
            }
        )
    return in_maps


def kernel(**inputs):
    global LAST_RESULT
    x = np.asarray(inputs["input"], dtype=np.float32)
    am = np.asarray(inputs["attention_mask"], dtype=np.float32)
    wq = np.asarray(inputs["wq"], dtype=np.float32)
    wk = np.asarray(inputs["wk"], dtype=np.float32)
    wv = np.asarray(inputs["wv"], dtype=np.float32)
    wo = np.asarray(inputs["wo"], dtype=np.float32)
    B, L, _ = x.shape

    nc = _get_program(L=L, QCH_=min(1024, L))
    in_maps = make_in_maps(x, am, wq, wk, wv, wo)
    trace = os.environ.get("KERNEL_TRACE", "0") == "1"
    res = run_bass_kernel_spmd(nc, in_maps, list(range(8)), trace=trace)
    LAST_RESULT = res

    out = np.zeros((B, L, 1024), dtype=np.float32)
    for b in range(B):
        for g in range(4):
            out[b] += res.results[4 * b + g]["out"].astype(np.float32)
    return out


# revision 65
# speedup vs baseline: 1.6284x; 1.0100x over previous
"""Decoder self-attention on 8 TRN2 NeuronCores.

Sharding: data-parallel over batch (2) x tensor-parallel over heads (4 groups
of 4 heads).  Core c handles batch c//4, heads 4*(c%4) .. 4*(c%4)+3.

v3 design (TimelineSim-guided, precision-validated):
  - q/k projections in SCALED fp8e4m3 DoubleRow (host sends 2*x and 32*w so
    fp8's subnormal floor is never hit; the 64x logit scale rides through the
    exp's scale argument).  Everything else on the ctx data path is bf16:
    fp8 noise (~2.4% rms) anywhere in the v/expS path translates ~1:1 into
    output error because ctx is a near-cancelling attention average, and the
    2e-2 budget only affords it on the q/k path (costs ~1.5e-2 via attention
    weights; numpy-emulated total 1.66e-2).
  - exp on the Act engine over PAIRED k-strips ([128, 2, 512] PSUM pair
    tiles, one call per pair) writing bf16 SBUF tiles consumed per-strip by
    bf16 PV matmuls.
  - causal masking: ONE gpsimd.affine_select per diagonal pair (iota affine
    in partition/parity/column covers both triangles and the odd strip's
    above-diagonal block).
  - pad mask folded into host inputs (x rows zeroed, v_aug ones-column = m).
  - normalize: DVE reciprocal of the PV rowsum row, gpsimd partition
    broadcast, DVE multiply into bf16 ctxT.  Final chunk's final head runs
    piecewise per l-tile, WO output DMA batched per l-tile.
"""

import os

import ml_dtypes
import numpy as np

import concourse.bass as bass
import concourse.tile as tile
from concourse import bacc, mybir
from concourse.bass_utils import run_bass_kernel_spmd

f32 = mybir.dt.float32
bf16 = mybir.dt.bfloat16
fp8 = mybir.dt.float8e4
DR = mybir.MatmulPerfMode.DoubleRow
EXP = mybir.ActivationFunctionType.Exp

H = 1024          # hidden dim
WG = 256          # weight-column group per core (4 heads x 64)
NH = 4            # heads per core
HD = 64           # head dim
QCH = 512         # q chunk
XS = 2.0          # host scale on fp8 x
WS = 32.0         # host scale on fp8 wq/wk

_PROGRAM_CACHE = {}
LAST_RESULT = None


def build_program(L=2048, QCH_=1024, opts=None):
    opts = dict(opts or {})
    qk8 = opts.get("qk8", True)
    NLT = L // 128
    NQH = L // QCH
    nc = bacc.Bacc("TRN2", target_bir_lowering=False, debug=False)

    x8_d = nc.dram_tensor("xT8", [H, L], fp8, kind="ExternalInput").ap()
    x8l_d = nc.dram_tensor("xT8l", [H, L], fp8, kind="ExternalInput").ap()
    w8_d = nc.dram_tensor("w8", [H, 4 * WG], fp8, kind="ExternalInput").ap()
    wo_d = nc.dram_tensor("wo16", [WG, H], bf16, kind="ExternalInput").ap()
    mcol_d = nc.dram_tensor("mcol", [128, NLT * NH], bf16, kind="ExternalInput").ap()
    out_d = nc.dram_tensor("out", [L, H], bf16, kind="ExternalOutput").ap()

    from contextlib import ExitStack

    with ExitStack() as ctx:
        tc = ctx.enter_context(tile.TileContext(nc))
        persist = ctx.enter_context(tc.tile_pool(name="persist", bufs=1))
        x8_sb = persist.tile([128, 8, L], fp8, tag="x8")
        x8l_sb = persist.tile([128, 8, L], fp8, tag="x8l")
        w8_sb = persist.tile([128, 8, 4 * WG], fp8, tag="w8")
        wo_sb = persist.tile([128, 2, H], bf16, tag="wo")
        qT = persist.tile([128, 2, L], bf16, tag="qT")
        kT = persist.tile([128, 2, L], bf16, tag="kT")
        v_aug = persist.tile([128, NLT, NH * (HD + 1)], bf16, tag="vaug")
        ctxT = persist.tile([128, 2, L], bf16, tag="ctxT")

        # ---- DMAs (sync/SP queue, consumption order) ----
        # w8 column order: [wq_c2=0|wk_c2=0 (0:256) | wvh (256:512) |
        #                   wvl (512:768) | wq_c2=1|wk_c2=1 (768:1024)]
        w8_r = w8_d.rearrange("(c p) d -> p c d", p=128)
        x8_r = x8_d.rearrange("(c p) l -> p c l", p=128)
        x8l_r = x8l_d.rearrange("(c p) l -> p c l", p=128)
        ones_cols = v_aug.rearrange("p t (h j) -> p t h j", j=HD + 1)[
            :, :, :, HD : HD + 1
        ]
        mcol_r = mcol_d.rearrange("p (t h) -> p t h", h=NH)
        nc.sync.dma_start(out=w8_sb[:, :, 0:256], in_=w8_r[:, :, 0:256])
        for lc in range(NQH):
            nc.sync.dma_start(
                out=x8_sb[:, :, QCH * lc : QCH * lc + QCH],
                in_=x8_r[:, :, QCH * lc : QCH * lc + QCH],
            )
            if lc == 0:
                nc.sync.dma_start(
                    out=w8_sb[:, :, 256:1024], in_=w8_r[:, :, 256:1024]
                )
            nc.sync.dma_start(
                out=x8l_sb[:, :, QCH * lc : QCH * lc + QCH],
                in_=x8l_r[:, :, QCH * lc : QCH * lc + QCH],
            )
            if lc == 0:
                nc.sync.dma_start(
                    out=ones_cols[:, 0:4], in_=mcol_r[:, 0:4, :, None]
                )
            if lc == 1:
                nc.sync.dma_start(
                    out=ones_cols[:, 4:NLT], in_=mcol_r[:, 4:NLT, :, None]
                )
        nc.sync.dma_start(out=wo_sb, in_=wo_d.rearrange("(c p) d -> p c d", p=128))

        # ---- pools (PSUM: spair 2x2 + ctx 2x1 + scratch 2x1 = 8 banks) ----
        s_ps = ctx.enter_context(
            tc.tile_pool(name="s_ps", bufs=opts.get("s_bufs", 2), space="PSUM")
        )
        ctx_ps = ctx.enter_context(
            tc.tile_pool(name="ctx_ps", bufs=opts.get("ctx_bufs", 2), space="PSUM")
        )
        scratch = ctx.enter_context(
            tc.tile_pool(name="scratch", bufs=opts.get("scr_bufs", 2), space="PSUM")
        )
        expp = ctx.enter_context(
            tc.tile_pool(name="expp", bufs=opts.get("expp_bufs", 14))
        )
        rp = ctx.enter_context(tc.tile_pool(name="rp", bufs=2))
        rbp = ctx.enter_context(tc.tile_pool(name="rbp", bufs=2))
        outp = ctx.enter_context(tc.tile_pool(name="outp", bufs=opts.get("outp_bufs", 4)))

        fill0 = nc.gpsimd.to_reg(0.0)

        # PE p-state warmup: ~3us of throwaway matmuls so the real
        # projections start at full clock.
        dummy = persist.tile([128, 2, 128], fp8, tag="dummy")
        nc.gpsimd.memset(dummy, 0.0)
        wps = scratch.tile([128, QCH], f32, tag="scr", name="warm")
        for _ in range(opts.get("warm", 60)):
            nc.tensor.matmul(
                wps[:, 0:128], lhsT=dummy, rhs=dummy,
                start=True, stop=True, perf_mode=DR,
            )

        def qk_proj(c2, lc, which=(0, 1)):
            for wi, dst in ((0, qT), (1, kT)):
                if wi not in which:
                    continue
                ps = scratch.tile([128, QCH], f32, tag="scr", name="qkps")
                w0 = (768 if c2 else 0) + 128 * wi
                for p in range(4):
                    for n in range(2):
                        nc.tensor.matmul(
                            ps[:, 256 * n : 256 * n + 256],
                            lhsT=w8_sb[:, 2 * p : 2 * p + 2, w0 : w0 + 128],
                            rhs=x8_sb[
                                :, 2 * p : 2 * p + 2,
                                QCH * lc + 256 * n : QCH * lc + 256 * n + 256,
                            ],
                            start=(p == 0 and n == 0),
                            stop=(p == 3 and n == 1),
                            perf_mode=DR,
                        )
                if lc == 0 and c2 == 0:
                    # evac the 256:512 half first — the first attention pair
                    # (j=1) reads exactly those columns
                    nc.vector.tensor_copy(
                        out=dst[:, c2, 256:512], in_=ps[:, 256:512]
                    )
                    nc.vector.tensor_copy(out=dst[:, c2, 0:256], in_=ps[:, 0:256])
                else:
                    nc.vector.tensor_copy(
                        out=dst[:, c2, QCH * lc : QCH * lc + QCH], in_=ps
                    )

        def v_proj(lc):
            # v scaled by XS*WS; hi-lo fp8: xh*wh + xl*wh + xh*wl
            terms = [(x8_sb, 256), (x8l_sb, 256), (x8_sb, 512)]
            for lt in range(4 * lc, 4 * lc + 4):
                ps = scratch.tile([128, QCH], f32, tag="scr", name="vps")
                for ti, (xs, wv0) in enumerate(terms):
                    for p in range(4):
                        nc.tensor.matmul(
                            ps[:, 0:WG],
                            lhsT=xs[:, 2 * p : 2 * p + 2, 128 * lt : 128 * lt + 128],
                            rhs=w8_sb[:, 2 * p : 2 * p + 2, wv0 : wv0 + WG],
                            start=(ti == 0 and p == 0),
                            stop=(ti == 2 and p == 3),
                            perf_mode=DR,
                        )
                dest = v_aug[:, lt, :].rearrange("p (h j) -> p h j", j=HD + 1)[
                    :, :, 0:HD
                ]
                nc.vector.tensor_copy(
                    out=dest, in_=ps[:, 0:WG].rearrange("p (h j) -> p h j", j=HD)
                )

        # ---- attention units ----
        # Within a head, process the two diagonal pairs FIRST so their
        # Pool-engine affine masks are off the end-of-head critical path.
        units = []
        for qh in range(NQH):
            jorder = [2 * qh + 1, 2 * qh] + list(range(2 * qh))
            for h in range(NH):
                for j in jorder:
                    units.append((qh, h, j))
        N = len(units)

        exp_tiles = {}
        ctx_cur = {}
        pending_wo = []
        SDIV = 32.0 * (XS * WS) ** 2 if qk8 else 32.0

        def emit_se(i):
            qh, h, j = units[i]
            if qh == 0 and j == 2 * qh + 1:
                if h == 0:
                    qk_proj(0, 0)
                    v_proj(0)
                elif h == 1:
                    qk_proj(1, 0)
                    qk_proj(0, 1, which=(0,))
                elif h == 2:
                    qk_proj(0, 1, which=(1,))
                    v_proj(1)
                else:
                    qk_proj(1, 1)
            q0 = QCH * qh
            p0 = HD * (h % 2)
            c2 = h // 2
            o = 256 if j == 2 * qh + 1 else 0
            s = s_ps.tile([128, 2, QCH], f32, tag="spair", name="spair")
            for parity in range(2):
                kt = 2 * j + parity
                os_ = max(0, 128 * kt - q0)
                nc.tensor.matmul(
                    s[:, parity, os_:QCH],
                    lhsT=kT[p0 : p0 + HD, c2, 128 * kt : 128 * kt + 128],
                    rhs=qT[p0 : p0 + HD, c2, q0 + os_ : q0 + QCH],
                    start=True,
                    stop=True,
                )
            ep = expp.tile([128, 2, QCH], bf16, tag="ep", name="ep")
            nc.scalar.activation(
                out=ep[:, :, o:QCH], in_=s[:, :, o:QCH], func=EXP, scale=1.0 / SDIV
            )
            if j >= 2 * qh:  # diagonal pair: causal triangle via affine iota
                nc.gpsimd.affine_select(
                    out=ep[:, :, o : o + 256],
                    in_=ep[:, :, o : o + 256],
                    pattern=[[-128, 2], [1, 256]],
                    compare_op=mybir.AluOpType.is_ge,
                    fill=fill0,
                    base=0,
                    channel_multiplier=-1,
                )
            exp_tiles[i] = ep

        def wo_lt(l0, tail):
            osb = outp.tile([128, H], bf16, tag="osb", name="osb")
            for n2 in range(2):
                w = scratch.tile([128, 512], f32, tag="scr", name="wops")
                for cc in range(2):
                    nc.tensor.matmul(
                        w,
                        lhsT=ctxT[:, cc, l0 : l0 + 128],
                        rhs=wo_sb[:, cc, 512 * n2 : 512 * n2 + 512],
                        start=(cc == 0),
                        stop=(cc == 1),
                    )
                if tail:
                    # Act is idle in the tail; keep DVE free for the
                    # normalize pieces and stream each half out immediately
                    nc.scalar.copy(out=osb[:, 512 * n2 : 512 * n2 + 512], in_=w)
                    nc.sync.dma_start(
                        out=out_d[l0 : l0 + 128, 512 * n2 : 512 * n2 + 512],
                        in_=osb[:, 512 * n2 : 512 * n2 + 512],
                    )
                else:
                    nc.vector.tensor_copy(
                        out=osb[:, 512 * n2 : 512 * n2 + 512], in_=w
                    )
            if not tail:
                nc.sync.dma_start(out=out_d[l0 : l0 + 128, :], in_=osb)

        def emit_pv(i):
            qh, h, j = units[i]
            q0 = QCH * qh
            p0 = HD * (h % 2)
            c2 = h // 2
            jfirst = 2 * qh + 1
            jlast = 2 * qh - 1 if qh > 0 else 0
            if j == jfirst and 0 < qh < NQH - 1:
                # stage next chunk's projections in pieces so the PE absorbs
                # them without starving the Act exp stream
                if h == 0:
                    qk_proj(0, qh + 1, which=(0,))
                elif h == 1:
                    qk_proj(0, qh + 1, which=(1,))
                elif h == 2:
                    v_proj(qh + 1)
                else:
                    qk_proj(1, qh + 1)
            if pending_wo:
                wo_lt(pending_wo.pop(0), tail=False)
            if j == jfirst:
                ctx_cur[h] = ctx_ps.tile([128, QCH], f32, tag="ctx", name="ctx")
            ctx_t = ctx_cur[h]
            ep = exp_tiles.pop(i)
            for parity in range(2):
                kt = 2 * j + parity
                os_ = max(0, 128 * kt - q0)
                nc.tensor.matmul(
                    ctx_t[0 : HD + 1, os_:QCH],
                    lhsT=v_aug[:, kt, (HD + 1) * h : (HD + 1) * (h + 1)],
                    rhs=ep[:, parity, os_:QCH],
                    start=(j == jfirst and parity == 0),
                    stop=(j == jlast and parity == 1),
                )
            if j != jlast:
                return
            tailpiece = qh == NQH - 1 and h == NH - 1
            if not tailpiece:
                r = rp.tile([1, QCH], f32, tag="r", name="r")
                nc.vector.reciprocal(r, ctx_t[HD : HD + 1, :])
                r64 = rbp.tile([HD, QCH], f32, tag="r64", name="r64")
                nc.gpsimd.partition_broadcast(r64, r)
                nc.vector.tensor_mul(
                    out=ctxT[p0 : p0 + HD, c2, q0 : q0 + QCH],
                    in0=ctx_t[0:HD, :],
                    in1=r64,
                )
                if h == NH - 1:
                    # spread this chunk's WO over the next chunk's pv units
                    pending_wo.extend(q0 + 128 * lt for lt in range(4))
            else:
                # final chunk's final head: piecewise normalize + WO to
                # minimize the serial tail
                for lt in range(4):
                    sl = slice(128 * lt, 128 * lt + 128)
                    rr = rp.tile([1, 128], f32, tag="rpc", name="rr")
                    nc.vector.reciprocal(rr, ctx_t[HD : HD + 1, sl])
                    r64 = rbp.tile([HD, 128], f32, tag="r64pc", name="r64pc")
                    nc.gpsimd.partition_broadcast(r64, rr)
                    nc.vector.tensor_mul(
                        out=ctxT[p0 : p0 + HD, c2, q0 + 128 * lt : q0 + 128 * lt + 128],
                        in0=ctx_t[0:HD, sl],
                        in1=r64,
                    )
                    wo_lt(q0 + 128 * lt, tail=True)

        LA = opts.get("lookahead", 12)
        se_cursor = 0
        for i in range(N):
            while se_cursor <= min(i + LA, N - 1):
                emit_se(se_cursor)
                se_cursor += 1
            emit_pv(i)

    nc.compile()
    return nc


def _get_program(L=2048, QCH_=1024):
    key = (L, QCH_)
    if key not in _PROGRAM_CACHE:
        _PROGRAM_CACHE[key] = build_program(L, QCH_)
    return _PROGRAM_CACHE[key]


def make_in_maps(x, am, wq, wk, wv, wo):
    B, L, _ = x.shape
    NLT = L // 128
    e4 = ml_dtypes.float8_e4m3
    bf = ml_dtypes.bfloat16
    in_maps = []
    for c in range(8):
        b, g = divmod(c, 4)
        cols = slice(WG * g, WG * (g + 1))
        m = am[b].astype(np.float32)
        xm = (x[b] * m[:, None]).astype(np.float32)
        wq_c, wk_c = wq[:, cols], wk[:, cols]
        # v weights: scaled hi-lo fp8 pair (wvh + wvl == 32*wv to ~bf16 acc)
        wvs = (WS * wv[:, cols]).astype(np.float32)
        wvh = wvs.astype(e4)
        wvl = (wvs - wvh.astype(np.float32)).astype(e4)
        w8 = np.concatenate(
            [
                (WS * wq_c[:, 0:128]).astype(e4),
                (WS * wk_c[:, 0:128]).astype(e4),
                wvh,
                wvl,
                (WS * wq_c[:, 128:256]).astype(e4),
                (WS * wk_c[:, 128:256]).astype(e4),
            ],
            axis=1,
        )
        # ones column carries XS*WS*m so the reciprocal absorbs the v scale
        mcol = np.repeat(
            (XS * WS * m).reshape(NLT, 128).T[:, :, None], NH, axis=2
        ).reshape(128, NLT * NH)
        xT = np.ascontiguousarray(xm.T)
        x8 = (XS * xT).astype(e4)
        x8l = (XS * xT - x8.astype(np.float32)).astype(e4)
        in_maps.append(
            {
                "xT8": x8,
                "xT8l": x8l,
                "w8": np.ascontiguousarray(w8),
                "wo16": np.ascontiguousarray(wo[cols, :]).astype(bf),
                "mcol": np.ascontiguousarray(mcol).astype(bf),
            }
        )
    return in_maps


def kernel(**inputs):
    global LAST_RESULT
    x = np.asarray(inputs["input"], dtype=np.float32)
    am = np.asarray(inputs["attention_mask"], dtype=np.float32)
    wq = np.asarray(inputs["wq"], dtype=np.float32)
    wk = np.asarray(inputs["wk"], dtype=np.float32)
    wv = np.asarray(inputs["wv"], dtype=np.float32)
    wo = np.asarray(inputs["wo"], dtype=np.float32)
    B, L, _ = x.shape

    nc = _get_program(L=L, QCH_=min(1024, L))
    in_maps = make_in_maps(x, am, wq, wk, wv, wo)
    trace = os.environ.get("KERNEL_TRACE", "0") == "1"
    res = run_bass_kernel_spmd(nc, in_maps, list(range(8)), trace=trace)
    LAST_RESULT = res

    out = np.zeros((B, L, 1024), dtype=np.float32)
    for b in range(B):
        for g in range(4):
            out[b] += res.results[4 * b + g]["out"].astype(np.float32)
    return out
